# revision 14
# baseline (speedup 1.0000x reference)
"""Multi-head causal attention with RoPE (B=1, S=4096, D=1024, H=16) on 8
Trainium2 NeuronCores.

Sharding: tensor-parallel over heads — each core computes 2 heads (QKV
projections column-sliced, attention, and its rank-128 partial of the output
projection; host sums the 8 partials = row-parallel wo).

On-chip layout is fully transposed ([feature, seq]) so the PE contracts over
the partition dim at every stage with no activation transposes:
  - x.T uploaded host-side; q.T/k.T/v.T = W_slice @ x.T
  - RoPE = (Wq x.T)*cosP + (Wq_swap x.T)*sinP with host-swapped/negated
    weight rows (elementwise only, no partition shifts)
  - scores.T[sk,sq] = matmul(lhsT=kT[64,128], rhs=qT[64,512]) per head
    (heads at partition 0:64 / 64:128 -> concurrent PE row-groups)
  - softmax: exp on ACT (1/sqrt(hd) folded into activation scale);
    denominators from an extra ones-column appended to V
  - attn@V: matmul(lhsT=[V|1][128sk,65], rhs=expT[128sk,512]) accumulated
    over sk tiles; row 64 = denominator; normalize via DVE mul with a
    PE-broadcast reciprocal row
  - O-proj: matmul(lhsT=woT_slice[128g,128dout], rhs=attnT) -> partial.T
Causality: above-diagonal 128x512 score tiles are skipped. The 4
diagonal-straddling tiles per 512-chunk are identical across chunks, loaded
once (values taken from the real mask input, pre-scaled x8 so the 1/8 exp
scale reproduces additive-mask semantics exactly). Projections and
attention interleave per chunk so ACT/DVE/DMA overlap PE from the start.
"""
import numpy as np

import concourse.bass as bass
import concourse.mybir as mybir
import concourse.tile as tile
from concourse.bass_utils import run_bass_kernel_spmd

B, S, D, H = 1, 4096, 1024, 16
HD = D // H            # 64
NC = 8                 # cores
HPC = H // NC          # 2 heads per core
SQC = 512              # seq chunk (matmul free dim)
NJ = S // SQC          # 8 chunks
NKT = S // 128         # 32 sk partition tiles
KT = D // 128          # 8 contraction tiles for projections

F32 = mybir.dt.float32
F32R = mybir.dt.float32r   # TF32-like: 1 cyc/row matmul at N>=256
MM_DT = F32R
AF = mybir.ActivationFunctionType

_MAX_WAITS = 1


def _fix_waits(nc):
    """walrus in this container rejects >1 sync-wait per instruction
    ("Too many sync wait commands"); split excess waits onto preceding
    same-engine NoOps (engine blocks in order, semantics preserved)."""
    n = 0
    for fn in nc.m.functions:
        for bb in fn.blocks:
            new_list = []
            for inst in bb.instructions:
                si = getattr(inst, "sync_info", None)
                if si is not None and si.on_wait and len(si.on_wait) > _MAX_WAITS:
                    waits = list(si.on_wait)
                    excess, keep = waits[:-_MAX_WAITS], waits[-_MAX_WAITS:]
                    for j in range(0, len(excess), _MAX_WAITS):
                        nop = mybir.InstNoOp(
                            name=f"I-waitfix-{nc.next_id()}",
                            ins=[],
                            outs=[],
                            engine=inst.engine,
                            sync_info=mybir.SyncInfo(
                                on_wait=excess[j : j + _MAX_WAITS], on_update=[]
                            ),
                        )
                        nc.register_instruction(nop)
                        new_list.append(nop)
                        n += 1
                    si.on_wait = keep
                new_list.append(inst)
            bb.instructions[:] = new_list
    return n


def build_program(mode: str, mm_dt=MM_DT, reps: int = 1, opts=None):
    """mode: 'causal' (skip above-diag tiles; one shared diag mask block),
    'zeros' (no mask), 'general' (full mask added on every tile)."""
    causal = mode == "causal"
    o = {"interleave": True, "oproj_act": False, "ex_bufs": 6, "sc_bufs": 3,
         "pp_bufs": 2, "oo_bufs": 3, "xc_bufs": 2, "phases": "all",
         "opp_bufs": 1, "trp_op": True}
    if opts:
        o.update(opts)
    nc = bass.Bass()

    xT_d = nc.dram_tensor("xT", (D, S), mm_dt, kind="ExternalInput")
    w_d = {
        n: nc.dram_tensor(n, (D, 128), mm_dt, kind="ExternalInput")
        for n in ("wq", "wqs", "wk", "wks", "wv")
    }
    wo_d = nc.dram_tensor("wo", (128, D), mm_dt, kind="ExternalInput")
    cos_d = nc.dram_tensor("cosP", (128, S), F32, kind="ExternalInput")
    sin_d = nc.dram_tensor("sinP", (128, S), F32, kind="ExternalInput")
    eye_d = nc.dram_tensor("eye2", (128, 64), F32, kind="ExternalInput")
    if causal:
        mask_d = nc.dram_tensor("maskd", (SQC, SQC), F32, kind="ExternalInput")
    elif mode == "general":
        mask_d = nc.dram_tensor("maskd", (NJ, S, SQC), F32, kind="ExternalInput")
    else:
        mask_d = None
    out_d = nc.dram_tensor("opT", (D, S), F32, kind="ExternalOutput")

    with tile.TileContext(nc) as tc:
        with (
            tc.tile_pool(name="wts", bufs=1) as wts,
            tc.tile_pool(name="big", bufs=1) as big,
            tc.tile_pool(name="xc", bufs=o["xc_bufs"]) as xcp,
            tc.tile_pool(name="cs", bufs=2) as csp,
            tc.tile_pool(name="rp", bufs=2) as rpp,
            tc.tile_pool(name="ex", bufs=o["ex_bufs"]) as exp_p,
            tc.tile_pool(name="mk", bufs=3) as mkp,
            tc.tile_pool(name="af", bufs=2) as afp,
            tc.tile_pool(name="tm", bufs=2) as tmp_p,
            tc.tile_pool(name="oo", bufs=o["oo_bufs"]) as oop,
            tc.tile_pool(name="rc", bufs=2) as rcp,
            tc.tile_pool(name="bc", bufs=2) as bcp_p,
            tc.tile_pool(name="pp", bufs=o["pp_bufs"], space=bass.MemorySpace.PSUM) as ppp,
            tc.tile_pool(name="sc", bufs=o["sc_bufs"], space=bass.MemorySpace.PSUM) as scp,
            tc.tile_pool(name="at0", bufs=1, space=bass.MemorySpace.PSUM) as at0p,
            tc.tile_pool(name="at1", bufs=1, space=bass.MemorySpace.PSUM) as at1p,
            tc.tile_pool(name="opp", bufs=max(1, o["opp_bufs"]), space=bass.MemorySpace.PSUM) as oppp,
        ):
            # ---- weights / constants ----
            w_sb = {}
            for n in ("wq", "wqs", "wk", "wks", "wv"):
                t = wts.tile([128, KT, 128], mm_dt, tag=n, name="t")
                for k in range(KT):
                    nc.sync.dma_start(t[:, k, :], w_d[n][k * 128 : (k + 1) * 128, :])
                w_sb[n] = t
            wo_sb = wts.tile([128, D], mm_dt, tag="wo", name="wo_sb")
            nc.sync.dma_start(wo_sb[:], wo_d[:])
            eye_sb = wts.tile([128, 64], F32, tag="eye", name="eye_sb")
            nc.sync.dma_start(eye_sb[:], eye_d[:])
            ones_sb = wts.tile([1, 64], F32R, tag="ones", name="ones_sb")
            onesf = wts.tile([1, 64], F32, tag="onesf", name="onesf")
            nc.vector.memset(onesf[:], 1.0)
            nc.vector.tensor_copy(ones_sb[:], onesf[:])

            qrot = big.tile([128, S], mm_dt, tag="qrot", name="qrot")
            krot = big.tile([128, S], mm_dt, tag="krot", name="krot")
            vT = big.tile([128, S], F32, tag="vT", name="vT")
            vext = big.tile([128, HPC * NKT, 65], mm_dt, tag="vext", name="vext")
            ones64 = wts.tile([128, HPC * NKT], F32, tag="ones64", name="ones64")
            nc.vector.memset(ones64[:], 1.0)
            nc.vector.tensor_copy(vext[:, :, 64], ones64[:])

            mdiag = None
            if causal:
                mdiag = wts.tile([128, 4, SQC], F32, tag="mdiag", name="mdiag")
                for r in range(4):
                    nc.sync.dma_start(
                        mdiag[:, r, :], mask_d[r * 128 : (r + 1) * 128, :]
                    )

            def proj(wname, xc):
                ps = ppp.tile([128, SQC], F32, tag="pp", name="ps")
                for k in range(KT):
                    nc.tensor.matmul(
                        ps[:],
                        w_sb[wname][:, k, :],
                        xc[:, k, :],
                        start=(k == 0),
                        stop=(k == KT - 1),
                    )
                return ps

            def do_proj_chunk(j):
                sl = slice(j * SQC, (j + 1) * SQC)
                xc = xcp.tile([128, KT, SQC], mm_dt, tag="xc", name="xc")
                for k in range(KT):
                    nc.sync.dma_start(xc[:, k, :], xT_d[k * 128 : (k + 1) * 128, sl])
                cosc = csp.tile([128, SQC], F32, tag="cosc", name="cosc")
                sinc = csp.tile([128, SQC], F32, tag="sinc", name="sinc")
                nc.sync.dma_start(cosc[:], cos_d[:, sl])
                nc.sync.dma_start(sinc[:], sin_d[:, sl])
                for main_w, swap_w, dest in (("wq", "wqs", qrot), ("wk", "wks", krot)):
                    ps_m = proj(main_w, xc)
                    t1 = rpp.tile([128, SQC], F32, tag="t1", name="t1")
                    nc.vector.tensor_mul(t1[:], ps_m[:], cosc[:])
                    ps_s = proj(swap_w, xc)
                    t2 = rpp.tile([128, SQC], F32, tag="t2", name="t2")
                    nc.vector.tensor_mul(t2[:], ps_s[:], sinc[:])
                    nc.vector.tensor_add(dest[:, sl], t1[:], t2[:])
                ps_v = proj("wv", xc)
                nc.vector.tensor_copy(vT[:, sl], ps_v[:])

            def do_vext_tiles(i_lo, i_hi):
                for h in range(HPC):
                    for i in range(i_lo, i_hi):
                        if o["trp_op"] and o["opp_bufs"] > 0:
                            trp = oppp.tile([128, SQC], F32, tag="opp", name="trp")
                        else:
                            trp = scp.tile([128, SQC], F32, tag="scps", name="trp")
                        nc.tensor.transpose(
                            trp[:, 0:64],
                            vT[h * 64 : (h + 1) * 64, i * 128 : (i + 1) * 128],
                            eye_sb[h * 64 : (h + 1) * 64, :],
                        )
                        nc.vector.tensor_copy(
                            vext[:, h * NKT + i, 0:64], trp[:, 0:64]
                        )

            def do_attn_chunk(j):
                sl = slice(j * SQC, (j + 1) * SQC)
                nkt_j = 4 * (j + 1) if causal else NKT
                afin = afp.tile([128, SQC], mm_dt, tag="afin", name="afin")
                at_t0 = at0p.tile([65, SQC], F32, tag="at0", name="at_t0")
                at_t1 = at1p.tile([65, SQC], F32, tag="at1", name="at_t1")
                atp = [at_t0, at_t1]
                # software pipeline: emit scores(i)+exp(i), then attnV(i-1),
                # so PE has score work while ACT computes exp of the previous tile
                ex_pend = [None, None]

                def emit_scores(i):
                    msk = None
                    if causal and i >= 4 * j:
                        msk = mdiag[:, i - 4 * j, :]
                    elif mode == "general":
                        mt = mkp.tile([128, SQC], F32, tag="msk", name="mt")
                        nc.sync.dma_start(
                            mt[:], mask_d[j, i * 128 : (i + 1) * 128, :]
                        )
                        msk = mt[:]
                    for h in range(HPC):
                        hsl = slice(h * 64, (h + 1) * 64)
                        sps = scp.tile([128, SQC], F32, tag="scps", name="sps")
                        nc.tensor.matmul(
                            sps[:],
                            krot[hsl, i * 128 : (i + 1) * 128],
                            qrot[hsl, sl],
                            start=True,
                            stop=True,
                        )
                        if msk is not None:
                            nc.vector.tensor_add(sps[:], sps[:], msk)
                        ex = exp_p.tile([128, SQC], mm_dt, tag="ex", name="ex")
                        nc.scalar.activation(ex[:], sps[:], AF.Exp, scale=0.125)
                        ex_pend[h] = ex

                def emit_attnv(i, exs):
                    for h in range(HPC):
                        nc.tensor.matmul(
                            atp[h][:, :],
                            vext[:, h * NKT + i, :],
                            exs[h][:],
                            start=(i == 0),
                            stop=(i == nkt_j - 1),
                        )

                prev = None
                for i in range(nkt_j):
                    emit_scores(i)
                    cur = list(ex_pend)
                    if prev is not None:
                        emit_attnv(i - 1, prev)
                    prev = cur
                emit_attnv(nkt_j - 1, prev)
                # normalize: rows 0:64 attn@V, row 64 denominator
                for h in range(HPC):
                    rec = rcp.tile([128, SQC], F32R, tag="rec", name="rec")
                    with nc.allow_low_precision("f32r reciprocal of softmax denom"):
                        nc.vector.reciprocal(rec[64:65, :], atp[h][64:65, :])
                    rec0 = rcp.tile([1, SQC], F32R, tag="rec0", name="rec0")
                    nc.sync.dma_start(rec0[:], rec[64:65, :])
                    bcps = scp.tile([128, SQC], F32, tag="scps", name="bcps")
                    nc.tensor.matmul(
                        bcps[0:64, :], ones_sb[:], rec0[:], start=True, stop=True
                    )
                    bcs = bcp_p.tile([64, SQC], F32, tag="bcs", name="bcs")
                    nc.vector.tensor_copy(bcs[:], bcps[0:64, :])
                    if h == 0:
                        nc.vector.tensor_mul(afin[0:64, :], atp[0][0:64, :], bcs[:])
                    else:
                        tmph = tmp_p.tile([64, SQC], mm_dt, tag="tmph", name="tmph")
                        nc.vector.tensor_mul(tmph[:], atp[1][0:64, :], bcs[:])
                        nc.sync.dma_start(afin[64:128, :], tmph[:])
                # output projection: partial.T[dout, sq]
                for dt_i in range(KT):
                    if o["opp_bufs"] > 0:
                        op = oppp.tile([128, SQC], F32, tag="opp", name="op")
                    else:
                        op = ppp.tile([128, SQC], F32, tag="pp", name="op")
                    nc.tensor.matmul(
                        op[:],
                        wo_sb[:, dt_i * 128 : (dt_i + 1) * 128],
                        afin[:],
                        start=True,
                        stop=True,
                    )
                    os_t = oop.tile([128, SQC], F32, tag="oo", name="os_t")
                    if o["oproj_act"]:
                        nc.scalar.copy(os_t[:], op[:])
                    else:
                        nc.vector.tensor_copy(os_t[:], op[:])
                    nc.sync.dma_start(
                        out_d[dt_i * 128 : (dt_i + 1) * 128, sl], os_t[:]
                    )

            for _rep in range(reps):
                if causal and o["interleave"]:
                    # interleaved: attention chunk j only needs k/v chunks <= j
                    for j in range(NJ):
                        do_proj_chunk(j)
                        do_vext_tiles(4 * j, 4 * j + 4)
                        do_attn_chunk(j)
                else:
                    ph = o["phases"]
                    for j in range(NJ):
                        do_proj_chunk(j)
                    if ph in ("all", "av"):
                        do_vext_tiles(0, NKT)
                    if ph == "all":
                        for j in range(NJ):
                            do_attn_chunk(j)

    _fix_waits(nc)
    return nc


def _host_prep(x, cos, sin, mask, wq, wk, wv, wo):
    xT = np.ascontiguousarray(x.reshape(S, D).T).astype(np.float32)

    idx = np.repeat(np.arange(HD // 2), 2)
    cosP_h = np.ascontiguousarray(np.asarray(cos)[:, idx].T)  # (64, S)
    sinP_h = np.ascontiguousarray(np.asarray(sin)[:, idx].T)
    cosP = np.vstack([cosP_h, cosP_h]).astype(np.float32)
    sinP = np.vstack([sinP_h, sinP_h]).astype(np.float32)

    eye2 = np.vstack([np.eye(64), np.eye(64)]).astype(np.float32)

    mask = np.asarray(mask)
    neg = np.isneginf(mask)
    triu = np.triu(np.ones((S, S), dtype=bool), 1)
    diag_ok = True
    blk0 = mask[0:SQC, 0:SQC]
    if neg.any():
        for j in range(1, NJ):
            b = mask[j * SQC : (j + 1) * SQC, j * SQC : (j + 1) * SQC]
            if not np.array_equal(b, blk0):
                diag_ok = False
                break
    if not neg.any() and not mask.any():
        mode = "zeros"
        maskd = None
    elif np.array_equal(neg, triu) and not mask[~neg].any() and diag_ok:
        mode = "causal"
        maskd = np.ascontiguousarray(blk0.T) * np.float32(8.0)
    else:
        mode = "general"
        maskd = np.empty((NJ, S, SQC), np.float32)
        for j in range(NJ):
            maskd[j] = mask[j * SQC : (j + 1) * SQC, :].T * np.float32(8.0)

    per_core = []
    for c in range(NC):
        hs, he = c * 128, (c + 1) * 128
        m = {
            "xT": xT,
            "cosP": cosP,
            "sinP": sinP,
            "eye2": eye2,
            "wo": np.ascontiguousarray(np.asarray(wo)[:, hs:he].T).astype(np.float32),
        }
        for name, w in (("wq", wq), ("wk", wk)):
            ws = np.asarray(w)[hs:he, :].astype(np.float32)
            sw = np.empty_like(ws)
            sw[0::2] = -ws[1::2]
            sw[1::2] = ws[0::2]
            m[name] = np.ascontiguousarray(ws.T)
            m[name + "s"] = np.ascontiguousarray(sw.T)
        m["wv"] = np.ascontiguousarray(np.asarray(wv)[hs:he, :].T).astype(np.float32)
        if maskd is not None:
            m["maskd"] = maskd
        per_core.append(m)
    return mode, per_core


_cache = {}


def kernel(x, cos, sin, mask, wq, wk, wv, wo, start_pos=0, **_):
    mode, in_maps = _host_prep(
        np.asarray(x), cos, sin, mask, np.asarray(wq), np.asarray(wk),
        np.asarray(wv), np.asarray(wo)
    )
    if mode not in _cache:
        _cache[mode] = build_program(mode)
    nc = _cache[mode]
    res = run_bass_kernel_spmd(nc, in_maps, core_ids=list(range(NC)))
    acc = np.zeros((D, S), np.float64)
    for c in range(NC):
        acc += res.results[c]["opT"].astype(np.float64)
    return np.ascontiguousarray(acc.T).reshape(B, S, D).astype(np.float32)


# revision 16
# speedup vs baseline: 34227.1702x; 34227.1702x over previous
"""Multi-head causal attention with RoPE (B=1, S=4096, D=1024, H=16) on 8
Trainium2 NeuronCores.

Sharding: tensor-parallel over heads — each core computes 2 heads (QKV
projections column-sliced, attention, and its rank-128 partial of the output
projection; host sums the 8 partials = row-parallel wo).

On-chip layout is fully transposed ([feature, seq]) so the PE contracts over
the partition dim at every stage with no activation transposes:
  - x.T uploaded host-side; q.T/k.T/v.T = W_slice @ x.T
  - RoPE = (Wq x.T)*cosP + (Wq_swap x.T)*sinP with host-swapped/negated
    weight rows (elementwise only, no partition shifts)
  - scores.T[sk,sq] = matmul(lhsT=kT[64,128], rhs=qT[64,512]) per head
    (heads at partition 0:64 / 64:128 -> concurrent PE row-groups)
  - softmax: exp on ACT (1/sqrt(hd) folded into activation scale);
    denominators from an extra ones-column appended to V
  - attn@V: matmul(lhsT=[V|1][128sk,65], rhs=expT[128sk,512]) accumulated
    over sk tiles; row 64 = denominator; normalize via DVE mul with a
    PE-broadcast reciprocal row
  - O-proj: matmul(lhsT=woT_slice[128g,128dout], rhs=attnT) -> partial.T
Causality: above-diagonal 128x512 score tiles are skipped. The 4
diagonal-straddling tiles per 512-chunk are identical across chunks, loaded
once (values taken from the real mask input, pre-scaled x8 so the 1/8 exp
scale reproduces additive-mask semantics exactly). Projections and
attention interleave per chunk so ACT/DVE/DMA overlap PE from the start.
"""
import numpy as np

import concourse.bass as bass
import concourse.mybir as mybir
import concourse.tile as tile
from concourse.bass_utils import run_bass_kernel_spmd

B, S, D, H = 1, 4096, 1024, 16
HD = D // H            # 64
NC = 8                 # cores
HPC = H // NC          # 2 heads per core
SQC = 512              # seq chunk (matmul free dim)
NJ = S // SQC          # 8 chunks
NKT = S // 128         # 32 sk partition tiles
KT = D // 128          # 8 contraction tiles for projections

F32 = mybir.dt.float32
F32R = mybir.dt.float32r   # TF32-like: 1 cyc/row matmul at N>=256
MM_DT = F32R
AF = mybir.ActivationFunctionType

_MAX_WAITS = 1


def _fix_waits(nc):
    """walrus in this container rejects >1 sync-wait per instruction
    ("Too many sync wait commands"); split excess waits onto preceding
    same-engine NoOps (engine blocks in order, semantics preserved)."""
    n = 0
    for fn in nc.m.functions:
        for bb in fn.blocks:
            new_list = []
            for inst in bb.instructions:
                si = getattr(inst, "sync_info", None)
                if si is not None and si.on_wait and len(si.on_wait) > _MAX_WAITS:
                    waits = list(si.on_wait)
                    excess, keep = waits[:-_MAX_WAITS], waits[-_MAX_WAITS:]
                    for j in range(0, len(excess), _MAX_WAITS):
                        nop = mybir.InstNoOp(
                            name=f"I-waitfix-{nc.next_id()}",
                            ins=[],
                            outs=[],
                            engine=inst.engine,
                            sync_info=mybir.SyncInfo(
                                on_wait=excess[j : j + _MAX_WAITS], on_update=[]
                            ),
                        )
                        nc.register_instruction(nop)
                        new_list.append(nop)
                        n += 1
                    si.on_wait = keep
                new_list.append(inst)
            bb.instructions[:] = new_list
    return n


def build_program(mode: str, mm_dt=MM_DT, reps: int = 1, opts=None):
    """mode: 'causal' (skip above-diag tiles; one shared diag mask block),
    'zeros' (no mask), 'general' (full mask added on every tile)."""
    causal = mode == "causal"
    o = {"interleave": True, "oproj_act": False, "ex_bufs": 6, "sc_bufs": 3,
         "pp_bufs": 2, "oo_bufs": 3, "xc_bufs": 2, "phases": "all",
         "opp_bufs": 1, "trp_op": True, "swpipe": 2, "oproj_split": True}
    if opts:
        o.update(opts)
    nc = bass.Bass()

    xT_d = nc.dram_tensor("xT", (D, S), mm_dt, kind="ExternalInput")
    w_d = {
        n: nc.dram_tensor(n, (D, 128), mm_dt, kind="ExternalInput")
        for n in ("wq", "wqs", "wk", "wks", "wv")
    }
    wo_d = nc.dram_tensor("wo", (128, D), mm_dt, kind="ExternalInput")
    cos_d = nc.dram_tensor("cosP", (128, S), F32, kind="ExternalInput")
    sin_d = nc.dram_tensor("sinP", (128, S), F32, kind="ExternalInput")
    eye_d = nc.dram_tensor("eye2", (128, 64), F32, kind="ExternalInput")
    if causal:
        mask_d = nc.dram_tensor("maskd", (SQC, SQC), F32, kind="ExternalInput")
    elif mode == "general":
        mask_d = nc.dram_tensor("maskd", (NJ, S, SQC), F32, kind="ExternalInput")
    else:
        mask_d = None
    out_d = nc.dram_tensor("opT", (D, S), F32, kind="ExternalOutput")

    with tile.TileContext(nc) as tc:
        with (
            tc.tile_pool(name="wts", bufs=1) as wts,
            tc.tile_pool(name="big", bufs=1) as big,
            tc.tile_pool(name="xc", bufs=o["xc_bufs"]) as xcp,
            tc.tile_pool(name="cs", bufs=2) as csp,
            tc.tile_pool(name="rp", bufs=2) as rpp,
            tc.tile_pool(name="ex", bufs=o["ex_bufs"]) as exp_p,
            tc.tile_pool(name="mk", bufs=3) as mkp,
            tc.tile_pool(name="af", bufs=2) as afp,
            tc.tile_pool(name="tm", bufs=2) as tmp_p,
            tc.tile_pool(name="oo", bufs=o["oo_bufs"]) as oop,
            tc.tile_pool(name="rc", bufs=2) as rcp,
            tc.tile_pool(name="bc", bufs=2) as bcp_p,
            tc.tile_pool(name="pp", bufs=o["pp_bufs"], space=bass.MemorySpace.PSUM) as ppp,
            tc.tile_pool(name="sc", bufs=o["sc_bufs"], space=bass.MemorySpace.PSUM) as scp,
            tc.tile_pool(name="at0", bufs=1, space=bass.MemorySpace.PSUM) as at0p,
            tc.tile_pool(name="at1", bufs=1, space=bass.MemorySpace.PSUM) as at1p,
            tc.tile_pool(name="opp", bufs=max(1, o["opp_bufs"]), space=bass.MemorySpace.PSUM) as oppp,
        ):
            # ---- weights / constants ----
            w_sb = {}
            for n in ("wq", "wqs", "wk", "wks", "wv"):
                t = wts.tile([128, KT, 128], mm_dt, tag=n, name="t")
                for k in range(KT):
                    nc.sync.dma_start(t[:, k, :], w_d[n][k * 128 : (k + 1) * 128, :])
                w_sb[n] = t
            wo_sb = wts.tile([128, D], mm_dt, tag="wo", name="wo_sb")
            nc.sync.dma_start(wo_sb[:], wo_d[:])
            wo_h1 = None
            if o["oproj_split"]:
                wo_h1 = wts.tile([64, D], mm_dt, tag="wo_h1", name="wo_h1")
                nc.sync.dma_start(wo_h1[:], wo_d[64:128, :])
            eye_sb = wts.tile([128, 64], F32, tag="eye", name="eye_sb")
            nc.sync.dma_start(eye_sb[:], eye_d[:])
            ones_sb = wts.tile([1, 64], F32R, tag="ones", name="ones_sb")
            onesf = wts.tile([1, 64], F32, tag="onesf", name="onesf")
            nc.vector.memset(onesf[:], 1.0)
            nc.vector.tensor_copy(ones_sb[:], onesf[:])

            qrot = big.tile([128, S], mm_dt, tag="qrot", name="qrot")
            krot = big.tile([128, S], mm_dt, tag="krot", name="krot")
            vT = big.tile([128, S], F32, tag="vT", name="vT")
            vext = big.tile([128, HPC * NKT, 65], mm_dt, tag="vext", name="vext")
            ones64 = wts.tile([128, HPC * NKT], F32, tag="ones64", name="ones64")
            nc.vector.memset(ones64[:], 1.0)
            nc.vector.tensor_copy(vext[:, :, 64], ones64[:])

            mdiag = None
            if causal:
                mdiag = wts.tile([128, 4, SQC], F32, tag="mdiag", name="mdiag")
                for r in range(4):
                    nc.sync.dma_start(
                        mdiag[:, r, :], mask_d[r * 128 : (r + 1) * 128, :]
                    )

            def proj(wname, xc):
                ps = ppp.tile([128, SQC], F32, tag="pp", name="ps")
                for k in range(KT):
                    nc.tensor.matmul(
                        ps[:],
                        w_sb[wname][:, k, :],
                        xc[:, k, :],
                        start=(k == 0),
                        stop=(k == KT - 1),
                    )
                return ps

            def do_proj_chunk(j):
                sl = slice(j * SQC, (j + 1) * SQC)
                xc = xcp.tile([128, KT, SQC], mm_dt, tag="xc", name="xc")
                for k in range(KT):
                    nc.sync.dma_start(xc[:, k, :], xT_d[k * 128 : (k + 1) * 128, sl])
                cosc = csp.tile([128, SQC], F32, tag="cosc", name="cosc")
                sinc = csp.tile([128, SQC], F32, tag="sinc", name="sinc")
                nc.sync.dma_start(cosc[:], cos_d[:, sl])
                nc.sync.dma_start(sinc[:], sin_d[:, sl])
                for main_w, swap_w, dest in (("wq", "wqs", qrot), ("wk", "wks", krot)):
                    ps_m = proj(main_w, xc)
                    t1 = rpp.tile([128, SQC], F32, tag="t1", name="t1")
                    nc.vector.tensor_mul(t1[:], ps_m[:], cosc[:])
                    ps_s = proj(swap_w, xc)
                    t2 = rpp.tile([128, SQC], F32, tag="t2", name="t2")
                    nc.vector.tensor_mul(t2[:], ps_s[:], sinc[:])
                    nc.vector.tensor_add(dest[:, sl], t1[:], t2[:])
                ps_v = proj("wv", xc)
                nc.vector.tensor_copy(vT[:, sl], ps_v[:])

            def do_vext_tiles(i_lo, i_hi):
                for h in range(HPC):
                    for i in range(i_lo, i_hi):
                        if o["trp_op"] and o["opp_bufs"] > 0:
                            trp = oppp.tile([128, SQC], F32, tag="opp", name="trp")
                        else:
                            trp = scp.tile([128, SQC], F32, tag="scps", name="trp")
                        nc.tensor.transpose(
                            trp[:, 0:64],
                            vT[h * 64 : (h + 1) * 64, i * 128 : (i + 1) * 128],
                            eye_sb[h * 64 : (h + 1) * 64, :],
                        )
                        nc.vector.tensor_copy(
                            vext[:, h * NKT + i, 0:64], trp[:, 0:64]
                        )

            def do_attn_chunk(j):
                sl = slice(j * SQC, (j + 1) * SQC)
                nkt_j = 4 * (j + 1) if causal else NKT
                afin = afp.tile([128, SQC], mm_dt, tag="afin", name="afin")
                at_t0 = at0p.tile([65, SQC], F32, tag="at0", name="at_t0")
                at_t1 = at1p.tile([65, SQC], F32, tag="at1", name="at_t1")
                atp = [at_t0, at_t1]
                # software pipeline: emit scores(i)+exp(i), then attnV(i-1),
                # so PE has score work while ACT computes exp of the previous tile
                ex_pend = [None, None]

                def emit_scores(i):
                    msk = None
                    if causal and i >= 4 * j:
                        msk = mdiag[:, i - 4 * j, :]
                    elif mode == "general":
                        mt = mkp.tile([128, SQC], F32, tag="msk", name="mt")
                        nc.sync.dma_start(
                            mt[:], mask_d[j, i * 128 : (i + 1) * 128, :]
                        )
                        msk = mt[:]
                    for h in range(HPC):
                        hsl = slice(h * 64, (h + 1) * 64)
                        sps = scp.tile([128, SQC], F32, tag="scps", name="sps")
                        nc.tensor.matmul(
                            sps[:],
                            krot[hsl, i * 128 : (i + 1) * 128],
                            qrot[hsl, sl],
                            start=True,
                            stop=True,
                        )
                        if msk is not None:
                            nc.vector.tensor_add(sps[:], sps[:], msk)
                        ex = exp_p.tile([128, SQC], mm_dt, tag="ex", name="ex")
                        nc.scalar.activation(ex[:], sps[:], AF.Exp, scale=0.125)
                        ex_pend[h] = ex

                def emit_attnv(i, exs):
                    for h in range(HPC):
                        nc.tensor.matmul(
                            atp[h][:, :],
                            vext[:, h * NKT + i, :],
                            exs[h][:],
                            start=(i == 0),
                            stop=(i == nkt_j - 1),
                        )

                depth = o["swpipe"]
                pend = []
                for i in range(nkt_j):
                    emit_scores(i)
                    pend.append((i, list(ex_pend)))
                    if len(pend) > depth:
                        ii, exs = pend.pop(0)
                        emit_attnv(ii, exs)
                for ii, exs in pend:
                    emit_attnv(ii, exs)
                # normalize: rows 0:64 attn@V, row 64 denominator
                af_h1 = [None]
                for h in range(HPC):
                    rec = rcp.tile([128, SQC], F32R, tag="rec", name="rec")
                    with nc.allow_low_precision("f32r reciprocal of softmax denom"):
                        nc.vector.reciprocal(rec[64:65, :], atp[h][64:65, :])
                    rec0 = rcp.tile([1, SQC], F32R, tag="rec0", name="rec0")
                    nc.sync.dma_start(rec0[:], rec[64:65, :])
                    bcps = scp.tile([128, SQC], F32, tag="scps", name="bcps")
                    nc.tensor.matmul(
                        bcps[0:64, :], ones_sb[:], rec0[:], start=True, stop=True
                    )
                    bcs = bcp_p.tile([64, SQC], F32, tag="bcs", name="bcs")
                    nc.vector.tensor_copy(bcs[:], bcps[0:64, :])
                    if h == 0:
                        nc.vector.tensor_mul(afin[0:64, :], atp[0][0:64, :], bcs[:])
                    elif o["oproj_split"]:
                        tmph = tmp_p.tile([64, SQC], mm_dt, tag="tmph", name="tmph")
                        nc.vector.tensor_mul(tmph[:], atp[1][0:64, :], bcs[:])
                        af_h1[0] = tmph
                    else:
                        tmph = tmp_p.tile([64, SQC], mm_dt, tag="tmph", name="tmph")
                        nc.vector.tensor_mul(tmph[:], atp[1][0:64, :], bcs[:])
                        nc.sync.dma_start(afin[64:128, :], tmph[:])
                # output projection: partial.T[dout, sq]
                for dt_i in range(KT):
                    if o["opp_bufs"] > 0:
                        op = oppp.tile([128, SQC], F32, tag="opp", name="op")
                    else:
                        op = ppp.tile([128, SQC], F32, tag="pp", name="op")
                    if o["oproj_split"]:
                        nc.tensor.matmul(
                            op[:],
                            wo_sb[0:64, dt_i * 128 : (dt_i + 1) * 128],
                            afin[0:64, :],
                            start=True,
                            stop=False,
                        )
                        nc.tensor.matmul(
                            op[:],
                            wo_h1[:, dt_i * 128 : (dt_i + 1) * 128],
                            af_h1[0][:],
                            start=False,
                            stop=True,
                        )
                    else:
                        nc.tensor.matmul(
                            op[:],
                            wo_sb[:, dt_i * 128 : (dt_i + 1) * 128],
                            afin[:],
                            start=True,
                            stop=True,
                        )
                    os_t = oop.tile([128, SQC], F32, tag="oo", name="os_t")
                    if o["oproj_act"]:
                        nc.scalar.copy(os_t[:], op[:])
                    else:
                        nc.vector.tensor_copy(os_t[:], op[:])
                    nc.sync.dma_start(
                        out_d[dt_i * 128 : (dt_i + 1) * 128, sl], os_t[:]
                    )

            for _rep in range(reps):
                if causal and o["interleave"]:
                    # interleaved: attention chunk j only needs k/v chunks <= j
                    for j in range(NJ):
                        do_proj_chunk(j)
                        do_vext_tiles(4 * j, 4 * j + 4)
                        do_attn_chunk(j)
                else:
                    ph = o["phases"]
                    for j in range(NJ):
                        do_proj_chunk(j)
                    if ph in ("all", "av"):
                        do_vext_tiles(0, NKT)
                    if ph == "all":
                        for j in range(NJ):
                            do_attn_chunk(j)

    _fix_waits(nc)
    return nc


def _host_prep(x, cos, sin, mask, wq, wk, wv, wo):
    xT = np.ascontiguousarray(x.reshape(S, D).T).astype(np.float32)

    idx = np.repeat(np.arange(HD // 2), 2)
    cosP_h = np.ascontiguousarray(np.asarray(cos)[:, idx].T)  # (64, S)
    sinP_h = np.ascontiguousarray(np.asarray(sin)[:, idx].T)
    cosP = np.vstack([cosP_h, cosP_h]).astype(np.float32)
    sinP = np.vstack([sinP_h, sinP_h]).astype(np.float32)

    eye2 = np.vstack([np.eye(64), np.eye(64)]).astype(np.float32)

    mask = np.asarray(mask)
    neg = np.isneginf(mask)
    triu = np.triu(np.ones((S, S), dtype=bool), 1)
    diag_ok = True
    blk0 = mask[0:SQC, 0:SQC]
    if neg.any():
        for j in range(1, NJ):
            b = mask[j * SQC : (j + 1) * SQC, j * SQC : (j + 1) * SQC]
            if not np.array_equal(b, blk0):
                diag_ok = False
                break
    if not neg.any() and not mask.any():
        mode = "zeros"
        maskd = None
    elif np.array_equal(neg, triu) and not mask[~neg].any() and diag_ok:
        mode = "causal"
        maskd = np.ascontiguousarray(blk0.T) * np.float32(8.0)
    else:
        mode = "general"
        maskd = np.empty((NJ, S, SQC), np.float32)
        for j in range(NJ):
            maskd[j] = mask[j * SQC : (j + 1) * SQC, :].T * np.float32(8.0)

    per_core = []
    for c in range(NC):
        hs, he = c * 128, (c + 1) * 128
        m = {
            "xT": xT,
            "cosP": cosP,
            "sinP": sinP,
            "eye2": eye2,
            "wo": np.ascontiguousarray(np.asarray(wo)[:, hs:he].T).astype(np.float32),
        }
        for name, w in (("wq", wq), ("wk", wk)):
            ws = np.asarray(w)[hs:he, :].astype(np.float32)
            sw = np.empty_like(ws)
            sw[0::2] = -ws[1::2]
            sw[1::2] = ws[0::2]
            m[name] = np.ascontiguousarray(ws.T)
            m[name + "s"] = np.ascontiguousarray(sw.T)
        m["wv"] = np.ascontiguousarray(np.asarray(wv)[hs:he, :].T).astype(np.float32)
        if maskd is not None:
            m["maskd"] = maskd
        per_core.append(m)
    return mode, per_core


_cache = {}


def kernel(x, cos, sin, mask, wq, wk, wv, wo, start_pos=0, **_):
    mode, in_maps = _host_prep(
        np.asarray(x), cos, sin, mask, np.asarray(wq), np.asarray(wk),
        np.asarray(wv), np.asarray(wo)
    )
    if mode not in _cache:
        _cache[mode] = build_program(mode)
    nc = _cache[mode]
    res = run_bass_kernel_spmd(nc, in_maps, core_ids=list(range(NC)))
    acc = np.zeros((D, S), np.float64)
    for c in range(NC):
        acc += res.results[c]["opT"].astype(np.float64)
    return np.ascontiguousarray(acc.T).reshape(B, S, D).astype(np.float32)
